# revision 4
# baseline (speedup 1.0000x reference)
"""Trainium2 Bass kernel for additive (Bahdanau) multi-head attention.

Module: q/k/v linear projections -> per-head additive attention
  scores[b,q,k] = sum_a av_a * tanh((q@Aq.T)[q,a] + (k@Ak.T)[k,a])
  out = softmax(scores) @ v  -> concat heads -> @ Wo.T

Key idea: replace tanh with an odd degree-11 polynomial fit on the actual
range of qf+kf (fitted on host from the runtime inputs).  The score then
separates:
  scores[q,k] = sum_{a,m,n} E[m,n] av_a zq[q,a]^m zk[k,a]^n
              = Umix^T @ Kpow     (one 480-contraction matmul per head)
where zq = qf/Rq, zk = kf/Rk, E[m,n] = c_{m+n} C(m+n,m) Rq^m Rk^n.
This removes the 10.5M-element tanh + reduction per core entirely; the
kernel becomes a short chain of fp16 matmuls.

Power features are built packed into [120 = 3-powers x 40-feat] partition
blocks without any cross-partition DMA: selector matmuls replicate
(1, z, z^2) and z^3 into packed layout, then the block recurrence
B_b = B_{b-1} * z3_rep runs on DVE with aligned partitions.

Softmax needs no max-subtraction (|scores| <= ~1.6 by construction);
normalization is a single per-partition tensor_scalar_mul with 1/rowsum.

Sharding: 8 cores; core c handles batch b=c//2 and a 128-row half of the
query dim; pure-concat gather (same as baseline).
"""
import sys

sys.path.insert(0, "/opt/trn_rl_repo")

import numpy as np

import concourse.bass as bass
import concourse.tile as tile
from concourse import bacc, mybir

F32 = mybir.dt.float32
F16 = mybir.dt.float16

B, LQ, LK, D, H = 4, 256, 256, 512, 8
DH, AH, NQ = 64, 40, 128
DEG = 11
NP = DEG + 1          # 12 powers: 0..11
NBLK = 4              # 4 feature blocks of 120 = 3 powers x 40
# mix pairs (nb, mb) ascending nb; E block (mb,nb) nonzero iff mb+nb<=3
PAIRS = [(nb, mb) for nb in range(4) for mb in range(4 - nb)]


def build_program():
    nc = bacc.Bacc("TRN2", target_bir_lowering=False, debug=False)

    xqT = nc.dram_tensor("xqT", [128, 4, NQ], F16, kind="ExternalInput")
    xkT = nc.dram_tensor("xkT", [128, 4, LK], F16, kind="ExternalInput")
    xvT = nc.dram_tensor("xvT", [128, 4, LK], F16, kind="ExternalInput")
    aqwT = nc.dram_tensor("aqwT", [128, 4, 4, 104], F16, kind="ExternalInput")
    akwT = nc.dram_tensor("akwT", [128, 4, 4, 104], F16, kind="ExternalInput")
    wvT = nc.dram_tensor("wvT", [128, 4, D], F16, kind="ExternalInput")
    woT = nc.dram_tensor("woT", [128, 4, D], F16, kind="ExternalInput")
    eav = nc.dram_tensor("eav", [120, len(PAIRS), 120], F16, kind="ExternalInput")
    sel = nc.dram_tensor("sel", [104, 3, 120], F16, kind="ExternalInput")
    selrep = nc.dram_tensor("selrep", [104, 120], F16, kind="ExternalInput")
    ident = nc.dram_tensor("ident", [128, 128], F16, kind="ExternalInput")
    y = nc.dram_tensor("y", [NQ, D], F32, kind="ExternalOutput")

    with tile.TileContext(nc) as tc:
        with (
            tc.tile_pool(name="const", bufs=1) as cpool,
            tc.tile_pool(name="pairw", bufs=2) as pairp,
            tc.tile_pool(name="headw", bufs=2) as hp,
            tc.tile_pool(name="sm", bufs=2) as smp,
            tc.tile_pool(name="ps1", bufs=1, space=bass.MemorySpace.PSUM) as ps1,
        ):
            # ---- static loads (ordered: earliest-needed first) ----
            xqT_s = cpool.tile([128, 4, NQ], F16)
            nc.sync.dma_start(xqT_s[:], xqT.ap())
            aqwT_s = cpool.tile([128, 4, 4, 104], F16)
            nc.sync.dma_start(aqwT_s[:], aqwT.ap())
            xkT_s = cpool.tile([128, 4, LK], F16)
            nc.sync.dma_start(xkT_s[:], xkT.ap())
            akwT_s = cpool.tile([128, 4, 4, 104], F16)
            nc.sync.dma_start(akwT_s[:], akwT.ap())
            sel_s = cpool.tile([104, 3, 120], F16)
            nc.sync.dma_start(sel_s[:], sel.ap())
            selrep_s = cpool.tile([104, 120], F16)
            nc.sync.dma_start(selrep_s[:], selrep.ap())
            eav_s = cpool.tile([120, len(PAIRS), 120], F16)
            nc.sync.dma_start(eav_s[:], eav.ap())
            ident_s = cpool.tile([128, 128], F16)
            nc.sync.dma_start(ident_s[:], ident.ap())
            xvT_s = cpool.tile([128, 4, LK], F16)
            nc.sync.dma_start(xvT_s[:], xvT.ap())
            wvT_s = cpool.tile([128, 4, D], F16)
            nc.sync.dma_start(wvT_s[:], wvT.ap())
            woT_s = cpool.tile([128, 4, D], F16)
            nc.sync.dma_start(woT_s[:], woT.ap())

            ones_s = cpool.tile([104, LK], F16)
            nc.vector.memset(ones_s[:], 1.0)

            v_s = cpool.tile([128, 2, D], F16)
            outcat_s = cpool.tile([128, 4, NQ], F16)

            state = {}

            def proj(g):
                # zq/zk for head pair (2g, 2g+1): heads at partition bases 0/64
                ps_p = ps1.tile([104, 3 * NQ], F32, tag="proj")
                for c in range(4):
                    nc.tensor.matmul(
                        ps_p[:, 0:NQ], aqwT_s[:, g, c, :], xqT_s[:, c, :],
                        start=(c == 0), stop=(c == 3),
                    )
                for c in range(4):
                    nc.tensor.matmul(
                        ps_p[:, NQ:3 * NQ], akwT_s[:, g, c, :], xkT_s[:, c, :],
                        start=(c == 0), stop=(c == 3),
                    )
                z1 = pairp.tile([104, 3 * NQ], F16, tag="z1")
                z2 = pairp.tile([104, 3 * NQ], F16, tag="z2")
                z3 = pairp.tile([104, 3 * NQ], F16, tag="z3")
                with nc.allow_low_precision("fp16 feature path"):
                    nc.vector.tensor_copy(z1[:], ps_p[:])
                    nc.vector.tensor_mul(z2[:], z1[:], z1[:])
                    nc.vector.tensor_mul(z3[:], z2[:], z1[:])
                state[g] = (z1, z2, z3)

            def pack_mms(h):
                # selector matmuls: B0 = [ones; z; z^2] packed, z3rep
                g, base = h // 2, 64 * (h % 2)
                z1, z2, z3 = state[g]
                ps_pk = ps1.tile([120, 2, 3 * NQ], F32, tag="pack")
                sl = slice(base, base + 40)
                qs, ks = slice(0, NQ), slice(NQ, 3 * NQ)
                nc.tensor.matmul(ps_pk[:, 0, 0:NQ], sel_s[sl, 0, :],
                                 ones_s[sl, 0:NQ],
                                 start=True, stop=False)
                nc.tensor.matmul(ps_pk[:, 0, 0:NQ], sel_s[sl, 1, :], z1[sl, qs],
                                 start=False, stop=False)
                nc.tensor.matmul(ps_pk[:, 0, 0:NQ], sel_s[sl, 2, :], z2[sl, qs],
                                 start=False, stop=True)
                nc.tensor.matmul(ps_pk[:, 1, 0:NQ], selrep_s[sl, :], z3[sl, qs],
                                 start=True, stop=True)
                nc.tensor.matmul(ps_pk[:, 0, ks], sel_s[sl, 0, :],
                                 ones_s[sl, :],
                                 start=True, stop=False)
                nc.tensor.matmul(ps_pk[:, 0, ks], sel_s[sl, 1, :], z1[sl, ks],
                                 start=False, stop=False)
                nc.tensor.matmul(ps_pk[:, 0, ks], sel_s[sl, 2, :], z2[sl, ks],
                                 start=False, stop=True)
                nc.tensor.matmul(ps_pk[:, 1, ks], selrep_s[sl, :], z3[sl, ks],
                                 start=True, stop=True)
                return ps_pk

            def pack_chains(h, ps_pk):
                qpk = hp.tile([120, NBLK, NQ], F16, tag="qpk")
                kpk = hp.tile([120, NBLK, LK], F16, tag="kpk")
                z3q = hp.tile([120, NQ], F16, tag="z3q")
                z3k = hp.tile([120, LK], F16, tag="z3k")
                with nc.allow_low_precision("fp16 feature path"):
                    nc.vector.tensor_copy(qpk[:, 0, :], ps_pk[:, 0, 0:NQ])
                    nc.vector.tensor_copy(z3q[:], ps_pk[:, 1, 0:NQ])
                    for b in range(1, NBLK):
                        nc.vector.tensor_mul(qpk[:, b, :], qpk[:, b - 1, :], z3q[:])
                    nc.vector.tensor_copy(kpk[:, 0, :], ps_pk[:, 0, NQ:3 * NQ])
                    nc.vector.tensor_copy(z3k[:], ps_pk[:, 1, NQ:3 * NQ])
                    for b in range(1, NBLK):
                        nc.vector.tensor_mul(kpk[:, b, :], kpk[:, b - 1, :], z3k[:])
                return qpk, kpk

            def mix_scores(h, qpk, kpk):
                ps_um = ps1.tile([120, NBLK, NQ], F32, tag="um")
                umix = hp.tile([120, NBLK, NQ], F16, tag="umix")
                for t, (nb, mb) in enumerate(PAIRS):
                    nc.tensor.matmul(
                        ps_um[:, nb, :], eav_s[:, t, :], qpk[:, mb, :],
                        start=(mb == 0), stop=(mb == 3 - nb),
                    )
                    if mb == 3 - nb:
                        nc.scalar.copy(umix[:, nb, :], ps_um[:, nb, :])
                ps_s = ps1.tile([128, LK], F32, tag="s")
                for b in range(NBLK):
                    nc.tensor.matmul(
                        ps_s[:], umix[:, b, :], kpk[:, b, :],
                        start=(b == 0), stop=(b == NBLK - 1),
                    )
                return ps_s

            def softmax(h, ps_s):
                w_raw = smp.tile([128, LK], F16, tag="wr")
                rsum = smp.tile([128, 1], F32, tag="rs")
                nc.scalar.activation(
                    w_raw[:], ps_s[:], mybir.ActivationFunctionType.Exp,
                    accum_out=rsum[:, 0:1],
                )
                rinv = smp.tile([128, 1], F32, tag="ri")
                w_n = smp.tile([128, LK], F16, tag="wn")
                nc.vector.reciprocal(rinv[:], rsum[:])
                with nc.allow_low_precision("fp16 softmax weights"):
                    nc.vector.tensor_scalar_mul(w_n[:], w_raw[:], rinv[:, 0:1])
                return w_n

            def transpose_wv(h, w_n, ps_o):
                ps_wt = ps1.tile([128, 2, NQ], F16, tag="wt")
                wT = smp.tile([128, 2, NQ], F16, tag="wT")
                for t in range(2):
                    nc.tensor.matmul(
                        ps_wt[:, t, :], w_n[:, t * 128:(t + 1) * 128], ident_s[:],
                        is_transpose=True, start=True, stop=True,
                    )
                    nc.scalar.copy(wT[:, t, :], ps_wt[:, t, :])
                r0 = 64 * (h % 2)
                for t in range(2):
                    nc.tensor.matmul(
                        ps_o[r0:r0 + 64, :],
                        v_s[:, t, h * DH:(h + 1) * DH], wT[:, t, :],
                        start=(t == 0), stop=(t == 1),
                        tile_position=(0, r0),
                    )
                if h % 2 == 1:
                    with nc.allow_low_precision("fp16 outcat"):
                        nc.vector.tensor_copy(outcat_s[:, h // 2, :], ps_o[:])

            def v_proj():
                for t in range(2):
                    ps_v = ps1.tile([128, D], F32, tag="big")
                    for c in range(4):
                        nc.tensor.matmul(
                            ps_v[:], xvT_s[:, c, t * 128:(t + 1) * 128],
                            wvT_s[:, c, :],
                            start=(c == 0), stop=(c == 3),
                        )
                    with nc.allow_low_precision("fp16 v"):
                        nc.vector.tensor_copy(v_s[:, t, :], ps_v[:])

            # ---- main pipeline, 1-head skew for softmax/transpose ----
            prev = {}
            ps_o = None
            for h in range(H):
                if h % 2 == 0:
                    proj(h // 2)
                ps_pk = pack_mms(h)
                qpk, kpk = pack_chains(h, ps_pk)
                ps_s = mix_scores(h, qpk, kpk)
                w_n = softmax(h, ps_s)
                if h == 0:
                    v_proj()
                if h > 0:
                    transpose_wv(h - 1, prev["w_n"], prev["ps_o"])
                if h % 2 == 0:
                    ps_o = ps1.tile([128, NQ], F32, tag="o")
                prev = {"w_n": w_n, "ps_o": ps_o}
            transpose_wv(H - 1, prev["w_n"], prev["ps_o"])

            # ---- final: y = outcat^T @ Wo^T with head-pair packing ----
            ps_fin = ps1.tile([NQ, D], F32, tag="big")
            for pr in range(4):
                nc.tensor.matmul(
                    ps_fin[:], outcat_s[:, pr, :], woT_s[:, pr, :],
                    start=(pr == 0), stop=(pr == 3),
                )
            fin_s = smp.tile([NQ, D], F32, tag="fin")
            nc.vector.tensor_copy(fin_s[:], ps_fin[:])
            nc.sync.dma_start(y.ap(), fin_s[:])

    nc.compile()
    return nc


def _host_shared(inputs):
    """Polynomial fit + all core-independent constant tensors."""
    from numpy.polynomial import chebyshev as C
    from math import comb

    f32 = np.float32
    queries = np.asarray(inputs["queries"], f32)
    keys = np.asarray(inputs["keys"], f32)
    Wq, Wk = np.asarray(inputs["Wq"], f32), np.asarray(inputs["Wk"], f32)
    Aq, Ak = np.asarray(inputs["Aq"], f32), np.asarray(inputs["Ak"], f32)
    av = np.asarray(inputs["av"], f32)

    MQ = np.stack([Aq @ Wq[h * DH:(h + 1) * DH, :] for h in range(H)])  # (H,AH,D)
    MK = np.stack([Ak @ Wk[h * DH:(h + 1) * DH, :] for h in range(H)])
    qf = np.einsum("bld,had->blha", queries, MQ, optimize=True)
    kf = np.einsum("bld,had->blha", keys, MK, optimize=True)
    Rq = float(np.abs(qf).max()) * 1.02 + 1e-6
    Rk = float(np.abs(kf).max()) * 1.02 + 1e-6
    R = Rq + Rk
    xs = np.linspace(-R, R, 4001)
    cfit = C.Chebyshev.fit(xs, np.tanh(xs), DEG)
    coefs = C.cheb2poly(cfit.convert().coef)

    E = np.zeros((NP, NP))
    for m in range(NP):
        for n in range(NP):
            if m + n <= DEG:
                E[m, n] = coefs[m + n] * comb(m + n, m) * Rq**m * Rk**n

    # eav[(jm*40+a), t, (jn*40+a)] = E[3mb+jm, 3nb+jn] * av[a]
    eav = np.zeros((120, len(PAIRS), 120), f32)
    ar = np.arange(AH)
    for t, (nb, mb) in enumerate(PAIRS):
        for jm in range(3):
            for jn in range(3):
                eav[jm * 40 + ar, t, jn * 40 + ar] = E[3 * mb + jm, 3 * nb + jn] * av
    # selectors, dual-base (head at partition base 0 and 64)
    sel = np.zeros((104, 3, 120), f32)
    selrep = np.zeros((104, 120), f32)
    for base in (0, 64):
        for j in range(3):
            sel[base + ar, j, j * 40 + ar] = 1.0
            selrep[base + ar, j * 40 + ar] = 1.0

    # projection weights: scaled, chunked, head pairs at col base 0/64
    aqwT = np.zeros((128, 4, 4, 104), f32)
    akwT = np.zeros((128, 4, 4, 104), f32)
    for g in range(4):
        for c in range(4):
            rows = slice(c * 128, (c + 1) * 128)
            aqwT[:, g, c, 0:40] = (MQ[2 * g] / Rq).T[rows]
            aqwT[:, g, c, 64:104] = (MQ[2 * g + 1] / Rq).T[rows]
            akwT[:, g, c, 0:40] = (MK[2 * g] / Rk).T[rows]
            akwT[:, g, c, 64:104] = (MK[2 * g + 1] / Rk).T[rows]

    Wv, Wo = np.asarray(inputs["Wv"], f32), np.asarray(inputs["Wo"], f32)
    wvT = np.ascontiguousarray(Wv.T.reshape(4, 128, D).transpose(1, 0, 2))
    woT = np.ascontiguousarray(Wo.T.reshape(4, 128, D).transpose(1, 0, 2))

    h16 = np.float16
    return {
        "aqwT": aqwT.astype(h16), "akwT": akwT.astype(h16),
        "wvT": wvT.astype(h16), "woT": woT.astype(h16),
        "eav": eav.astype(h16), "sel": sel.astype(h16),
        "selrep": selrep.astype(h16),
        "ident": np.eye(128, dtype=h16),
    }


def host_prep(inputs, shared, core):
    b, qh = core // 2, core % 2
    qs = qh * NQ
    f32 = np.float32
    h16 = np.float16
    queries = np.asarray(inputs["queries"], f32)
    keys = np.asarray(inputs["keys"], f32)
    values = np.asarray(inputs["values"], f32)
    m = dict(shared)
    m["xqT"] = np.ascontiguousarray(
        queries[b, qs:qs + NQ, :].T.reshape(4, 128, NQ).transpose(1, 0, 2)
    ).astype(h16)
    m["xkT"] = np.ascontiguousarray(
        keys[b].T.reshape(4, 128, LK).transpose(1, 0, 2)).astype(h16)
    m["xvT"] = np.ascontiguousarray(
        values[b].T.reshape(4, 128, LK).transpose(1, 0, 2)).astype(h16)
    return m


_NC_CACHE = {}


def _get_nc():
    if "nc" not in _NC_CACHE:
        _NC_CACHE["nc"] = build_program()
    return _NC_CACHE["nc"]


def make_in_maps(**inputs):
    inputs = {k: np.asarray(v) for k, v in inputs.items()}
    shared = _host_shared(inputs)
    return [host_prep(inputs, shared, core) for core in range(8)]


def unshard(results):
    out = np.empty((B, LQ, D), dtype=np.float32)
    for core in range(8):
        b, qh = core // 2, core % 2
        out[b, qh * NQ:(qh + 1) * NQ, :] = results[core]["y"]
    return out


def kernel(**inputs) -> np.ndarray:
    from concourse.bass_utils import run_bass_kernel_spmd

    nc = _get_nc()
    in_maps = make_in_maps(**inputs)
    res = run_bass_kernel_spmd(nc, in_maps, core_ids=list(range(8)))
    return unshard(res.results)


if __name__ == "__main__":
    rng = np.random.default_rng(0)
    demo = {
        "queries": rng.standard_normal((B, LQ, D), dtype=np.float32),
        "keys": rng.standard_normal((B, LQ, D), dtype=np.float32),
        "values": rng.standard_normal((B, LQ, D), dtype=np.float32),
        "Wq": rng.standard_normal((D, D), dtype=np.float32) * 0.05,
        "Wk": rng.standard_normal((D, D), dtype=np.float32) * 0.05,
        "Wv": rng.standard_normal((D, D), dtype=np.float32) * 0.05,
        "Wo": rng.standard_normal((D, D), dtype=np.float32) * 0.05,
        "Aq": rng.standard_normal((AH, DH), dtype=np.float32) * 0.05,
        "Ak": rng.standard_normal((AH, DH), dtype=np.float32) * 0.05,
        "av": rng.standard_normal((AH,), dtype=np.float32) * 0.05,
    }
    out = kernel(**demo)
    print("kernel ran, output:", out.shape, out.dtype)


# revision 36
# speedup vs baseline: 1.2350x; 1.2350x over previous
"""Trainium2 Bass kernel for additive (Bahdanau) multi-head attention.

Module: q/k/v linear projections -> per-head additive attention
  scores[b,q,k] = sum_a av_a * tanh((q@Aq.T)[q,a] + (k@Ak.T)[k,a])
  out = softmax(scores) @ v  -> concat heads -> @ Wo.T

Key idea: replace tanh with an odd degree-11 polynomial fitted on the
actual runtime range of qf+kf (host-side Chebyshev fit).  The score then
separates into a plain bilinear form of power features:
  scores[q,k] = sum_{a,m,n} E[m,n] av_a zq[q,a]^m zk[k,a]^n
with zq = qf/Rq, zk = kf/Rk, E[m,n] = c_{m+n} C(m+n,m) Rq^m Rk^n.
This removes the 10.5M-element tanh + reduction per core entirely; the
kernel is a short chain of fp16 matmuls (LDWEIGHTS-bound, so matmuls are
batched: Q|K packed in one moving stream, mix batched over head pairs).

Power features are built packed in [120 = 3-powers x 40-feat] partition
blocks with no cross-partition traffic: selector matmuls place (z, z^2)
and replicate z^3, then the block recurrence B_b = B_{b-1} * z3_rep runs
on DVE with aligned partitions.  w^T comes from DMA-transpose (idle DMA
engines) instead of PE transposes.

Softmax needs no max-subtraction (|scores| <= ~1.6 by construction);
normalization is one per-partition tensor_scalar_mul with 1/rowsum.

Sharding: 8 cores; core c handles batch b=c//2 and a 128-row half of the
query dim; pure-concat gather.
"""
import os
import sys

sys.path.insert(0, "/opt/trn_rl_repo")

DMA_TRANSPOSE = os.environ.get("KERNEL_DMA_T", "1") == "1"
COLTILE_WV = os.environ.get("KERNEL_COLTILE", "1") == "1"

import numpy as np

import concourse.bass as bass
import concourse.tile as tile
from concourse import bacc, mybir

F32 = mybir.dt.float32
F16 = mybir.dt.float16

B, LQ, LK, D, H = 4, 256, 256, 512, 8
DH, AH, NQ = 64, 40, 128
QK = NQ + LK          # 384: q-cols | k-cols in one moving stream
DEG = 11
NP = DEG + 1
NBLK = 4              # feature blocks of 120 = 3 powers x 40
PAIRS = [(nb, mb) for nb in range(4) for mb in range(4 - nb)]


def build_program():
    nc = bacc.Bacc("TRN2", target_bir_lowering=False, debug=False)

    xqk = nc.dram_tensor("xqk", [128, 4, QK], F16, kind="ExternalInput")
    xvT = nc.dram_tensor("xvT", [128, 4, LK], F16, kind="ExternalInput")
    aqkw = nc.dram_tensor("aqkw", [128, 4, 4, 208], F16, kind="ExternalInput")
    wvT = nc.dram_tensor("wvT", [128, 4, D], F16, kind="ExternalInput")
    woT = nc.dram_tensor("woT", [128, 4, D], F16, kind="ExternalInput")
    eav = nc.dram_tensor("eav", [120, len(PAIRS), 120], F16, kind="ExternalInput")
    sel3 = nc.dram_tensor("sel3", [104, 3, 120], F16, kind="ExternalInput")
    ident = nc.dram_tensor("ident", [128, 128], F16, kind="ExternalInput")
    woT8 = nc.dram_tensor("woT8", [DH, H, D], F16, kind="ExternalInput")
    y = nc.dram_tensor("y", [NQ, D], F32, kind="ExternalOutput")

    with tile.TileContext(nc) as tc:
        with (
            tc.tile_pool(name="const", bufs=1) as cpool,
            tc.tile_pool(name="pairw", bufs=2) as pairp,
            tc.tile_pool(name="headw", bufs=2) as hp,
            tc.tile_pool(name="sm", bufs=2) as smp,
            tc.tile_pool(name="wtp", bufs=4) as wtp,
            tc.tile_pool(name="ps1", bufs=1, space=bass.MemorySpace.PSUM) as ps1,
        ):
            # ---- static loads (earliest-needed first, few large DMAs) ----
            xqk_s = cpool.tile([128, 4, QK], F16)
            aqkw_s = cpool.tile([128, 4, 4, 208], F16)
            nc.sync.dma_start(xqk_s[:], xqk.ap())
            nc.scalar.dma_start(aqkw_s[:, 0, :, :], aqkw.ap()[:, 0, :, :])
            nc.scalar.dma_start(aqkw_s[:, 1, :, :], aqkw.ap()[:, 1, :, :])
            sel3_s = cpool.tile([104, 3, 120], F16)
            nc.sync.dma_start(sel3_s[:], sel3.ap())
            nc.sync.dma_start(aqkw_s[:, 2, :, :], aqkw.ap()[:, 2, :, :])
            nc.sync.dma_start(aqkw_s[:, 3, :, :], aqkw.ap()[:, 3, :, :])
            eav_s = cpool.tile([120, len(PAIRS), 120], F16)
            nc.sync.dma_start(eav_s[:], eav.ap())
            xvT_s = cpool.tile([128, 4, LK], F16)
            wvT_s = cpool.tile([128, 4, D], F16)
            woT_s = cpool.tile([128, 4, D], F16)

            def late_loads():
                nc.scalar.dma_start(xvT_s[:], xvT.ap())
                nc.scalar.dma_start(wvT_s[:], wvT.ap())
                if COLTILE_WV:
                    nc.scalar.dma_start(woT_s[:], woT.ap())

            ident_s = cpool.tile([128, 128], F16)
            nc.sync.dma_start(ident_s[:], ident.ap())
            woT8_s = cpool.tile([DH, H, D], F16)
            if not COLTILE_WV:
                nc.sync.dma_start(woT8_s[:], woT8.ap())
            v_s = cpool.tile([128, 2, D], F16)
            outcat_s = cpool.tile([128, 4, NQ], F16)
            outcat8_s = cpool.tile([DH, H, NQ], F16)

            # HAM warm-up: the PE is idle for ~12us waiting on input DMAs,
            # which leaves the clock gate at 4/8 (1.2 GHz) for the first
            # real matmuls.  Fill the wait with dummy matmuls on a memset
            # scratch tile so the array is at 8/8 when real work arrives.
            # Output goes to the "o0" psum slot (first real use is ~30us in)
            # and is never read.
            scratch_s = cpool.tile([128, D], F16)
            nc.vector.memset(scratch_s[:], 0.5)
            wu_ps = ps1.tile([128, D], F32, tag="o0")
            for _ in range(12):
                nc.tensor.matmul(wu_ps[:], scratch_s[:, 0:128], scratch_s[:],
                                 start=True, stop=True)

            state = {}

            def proj(g):
                # zq|zk for head pair (2g, 2g+1) at partition bases 0/64
                ps_p = ps1.tile([104, QK], F32, tag="big")
                for c in range(4):
                    nc.tensor.matmul(
                        ps_p[:, 0:NQ], aqkw_s[:, g, c, 0:104],
                        xqk_s[:, c, 0:NQ],
                        start=(c == 0), stop=(c == 3),
                    )
                for c in range(4):
                    nc.tensor.matmul(
                        ps_p[:, NQ:QK], aqkw_s[:, g, c, 104:208],
                        xqk_s[:, c, NQ:QK],
                        start=(c == 0), stop=(c == 3),
                    )
                z1 = pairp.tile([104, QK], F16, tag="z1")
                z2 = pairp.tile([104, QK], F16, tag="z2")
                z3 = pairp.tile([104, QK], F16, tag="z3")
                with nc.allow_low_precision("fp16 feature path"):
                    nc.scalar.copy(z1[:], ps_p[:])
                    nc.vector.tensor_mul(z2[:], z1[:], z1[:])
                    nc.vector.tensor_mul(z3[:], z2[:], z1[:])
                state[g] = (z1, z2, z3)

            def pack(h, feat):
                # B0 = [ones; z; z^2] packed (ones row set after the copy),
                # z3rep: z^3 replicated to all three row groups
                g, base, hh = h // 2, 64 * (h % 2), h % 2
                z1, z2, z3 = state[g]
                # slot stride padded to a full PSUM bank (512 fp32) so no
                # matmul write crosses a bank boundary
                ps_pk = ps1.tile([120, 2, 512], F32, tag="pk")
                sl = slice(base, base + 40)
                nc.tensor.matmul(ps_pk[:, 0, 0:QK], sel3_s[sl, 0, :], z1[sl, :],
                                 start=True, stop=False)
                nc.tensor.matmul(ps_pk[:, 0, 0:QK], sel3_s[sl, 1, :], z2[sl, :],
                                 start=False, stop=True)
                nc.tensor.matmul(ps_pk[:, 1, 0:QK], sel3_s[sl, 2, :], z3[sl, :],
                                 start=True, stop=True)
                z3r = hp.tile([120, QK], F16, tag=f"z3r{hh}")
                with nc.allow_low_precision("fp16 feature path"):
                    nc.vector.tensor_copy(feat[:, 0, hh, :], ps_pk[:, 0, 0:QK])
                    nc.scalar.copy(z3r[:], ps_pk[:, 1, 0:QK])
                    nc.vector.memset(feat[0:40, 0, hh, :], 1.0)
                    for b in range(1, NBLK):
                        nc.vector.tensor_mul(feat[:, b, hh, :],
                                             feat[:, b - 1, hh, :], z3r[:])

            def mix_pair(feat):
                # Umix for both heads of the pair in one moving stream.
                # Two psum tiles from the shared "um" ring (scores reuses it).
                ps_a = ps1.tile([128, 2, 2 * NQ], F32, tag="um0")
                ps_b = ps1.tile([128, 2, 2 * NQ], F32, tag="um1")
                umix = hp.tile([120, NBLK, 2, NQ], F16, tag="umix")
                for t, (nb, mb) in enumerate(PAIRS):
                    ps_um = ps_a if nb < 2 else ps_b
                    nc.tensor.matmul(
                        ps_um[0:120, nb % 2, :], eav_s[:, t, :],
                        feat[:, mb, :, 0:NQ],
                        start=(mb == 0), stop=(mb == 3 - nb),
                    )
                    if mb == 3 - nb:
                        nc.scalar.copy(umix[:, nb, :, :], ps_um[0:120, nb % 2, :])
                return umix

            def scores(h, feat, umix):
                hh = h % 2
                ps_sf = ps1.tile([128, LK], F32, tag="s")
                ps_s = ps_sf[:]
                for b in range(NBLK):
                    nc.tensor.matmul(
                        ps_s[:], umix[:, b, hh, :], feat[:, b, hh, NQ:QK],
                        start=(b == 0), stop=(b == NBLK - 1),
                    )
                return ps_s

            def softmax(h, ps_s):
                w_raw = smp.tile([128, LK], F16, tag="wr")
                rsum = smp.tile([128, 1], F32, tag="rs")
                nc.scalar.activation(
                    w_raw[:], ps_s[:], mybir.ActivationFunctionType.Exp,
                    accum_out=rsum[:, 0:1],
                )
                rinv = smp.tile([128, 1], F32, tag="ri")
                w_n = smp.tile([128, LK], F16, tag="wn")
                nc.vector.reciprocal(rinv[:], rsum[:])
                with nc.allow_low_precision("fp16 softmax weights"):
                    nc.vector.tensor_scalar_mul(w_n[:], w_raw[:], rinv[:, 0:1])
                # transpose via DMA xbar (idle engines, frees PE);
                # drain-phase heads use the PE (idle there, no DMA latency)
                wT = wtp.tile([128, 2, NQ], F16, tag="wT")
                if DMA_TRANSPOSE:
                    for t in range(2):
                        nc.sync.dma_start(wT[:, t, :],
                                          w_n[:, t * 128:(t + 1) * 128],
                                          transpose=True)
                else:
                    # reuse the scores bank (freed once exp(h) has read it)
                    ps_wt = ps1.tile([128, 2, NQ], F16, tag="s")
                    for t in range(2):
                        nc.tensor.matmul(ps_wt[:, t, :],
                                         w_n[:, t * 128:(t + 1) * 128],
                                         ident_s[:], is_transpose=True,
                                         start=True, stop=True)
                        with nc.allow_low_precision("fp16 wT"):
                            nc.vector.tensor_copy(wT[:, t, :], ps_wt[:, t, :])
                return wT

            def wv(h, wT, ps_o):
                if COLTILE_WV:
                    r0 = 64 * (h % 2)
                    for t in range(2):
                        nc.tensor.matmul(
                            ps_o[r0:r0 + 64, :],
                            v_s[:, t, h * DH:(h + 1) * DH], wT[:, t, :],
                            start=(t == 0), stop=(t == 1),
                            tile_position=(0, r0),
                        )
                    if h % 2 == 1:
                        with nc.allow_low_precision("fp16 outcat"):
                            nc.vector.tensor_copy(outcat_s[:, h // 2, :],
                                                  ps_o[:])
                else:
                    ps_oh = ps1.tile([DH, NQ], F32, tag="o")
                    for t in range(2):
                        nc.tensor.matmul(
                            ps_oh[:],
                            v_s[:, t, h * DH:(h + 1) * DH], wT[:, t, :],
                            start=(t == 0), stop=(t == 1),
                        )
                    with nc.allow_low_precision("fp16 outcat"):
                        nc.vector.tensor_copy(outcat8_s[:, h, :], ps_oh[:])

            def v_proj():
                for t in range(2):
                    ps_v = ps1.tile([128, D], F32, tag="big")
                    for c in range(4):
                        nc.tensor.matmul(
                            ps_v[:], xvT_s[:, c, t * 128:(t + 1) * 128],
                            wvT_s[:, c, :],
                            start=(c == 0), stop=(c == 3),
                        )
                    with nc.allow_low_precision("fp16 v"):
                        nc.vector.tensor_copy(v_s[:, t, :], ps_v[:])

            # ---- main pipeline over head pairs, 2-head skew for wv ----
            pend = []  # (h, wT, ps_o)
            ps_o = None
            for g in range(4):
                proj(g)
                feat = hp.tile([120, NBLK, 2, QK], F16, tag="feat")
                pack(2 * g, feat)
                if g == 0:
                    late_loads()
                pack(2 * g + 1, feat)
                umix = mix_pair(feat)
                for hh in range(2):
                    h = 2 * g + hh
                    ps_s = scores(h, feat, umix)
                    wT = softmax(h, ps_s)
                    if h == 0:
                        v_proj()
                    if h % 2 == 0:
                        ps_o = ps1.tile([128, NQ], F32, tag="o" + str((h // 2) % 2))
                    pend.append((h, wT, ps_o))
                    if len(pend) > 2:
                        wv(*pend.pop(0))
            while pend:
                wv(*pend.pop(0))

            # ---- final: y = outcat^T @ Wo^T ----
            ps_fin = ps1.tile([NQ, D], F32, tag="big")
            if COLTILE_WV:
                for pr in range(4):
                    nc.tensor.matmul(
                        ps_fin[:], outcat_s[:, pr, :], woT_s[:, pr, :],
                        start=(pr == 0), stop=(pr == 3),
                    )
            else:
                for h in range(H):
                    nc.tensor.matmul(
                        ps_fin[:], outcat8_s[:, h, :], woT8_s[:, h, :],
                        start=(h == 0), stop=(h == H - 1),
                    )
            fin_s = smp.tile([NQ, D], F32, tag="fin")
            nc.vector.tensor_copy(fin_s[:], ps_fin[:])
            nc.sync.dma_start(y.ap(), fin_s[:])

    nc.compile()
    return nc


def _host_shared(inputs):
    """Polynomial fit + all core-independent constant tensors."""
    from numpy.polynomial import chebyshev as C
    from math import comb

    f32 = np.float32
    queries = np.asarray(inputs["queries"], f32)
    keys = np.asarray(inputs["keys"], f32)
    Wq, Wk = np.asarray(inputs["Wq"], f32), np.asarray(inputs["Wk"], f32)
    Aq, Ak = np.asarray(inputs["Aq"], f32), np.asarray(inputs["Ak"], f32)
    av = np.asarray(inputs["av"], f32)

    MQ = np.stack([Aq @ Wq[h * DH:(h + 1) * DH, :] for h in range(H)])  # (H,AH,D)
    MK = np.stack([Ak @ Wk[h * DH:(h + 1) * DH, :] for h in range(H)])
    qf = np.einsum("bld,had->blha", queries, MQ, optimize=True)
    kf = np.einsum("bld,had->blha", keys, MK, optimize=True)
    Rq = float(np.abs(qf).max()) * 1.02 + 1e-6
    Rk = float(np.abs(kf).max()) * 1.02 + 1e-6
    R = Rq + Rk
    xs = np.linspace(-R, R, 4001)
    cfit = C.Chebyshev.fit(xs, np.tanh(xs), DEG)
    coefs = C.cheb2poly(cfit.convert().coef)

    E = np.zeros((NP, NP))
    for m in range(NP):
        for n in range(NP):
            if m + n <= DEG:
                E[m, n] = coefs[m + n] * comb(m + n, m) * Rq**m * Rk**n

    eav = np.zeros((120, len(PAIRS), 120), f32)
    ar = np.arange(AH)
    for t, (nb, mb) in enumerate(PAIRS):
        for jm in range(3):
            for jn in range(3):
                eav[jm * 40 + ar, t, jn * 40 + ar] = E[3 * mb + jm, 3 * nb + jn] * av
    # selectors for (z, z^2) placement + z^3 replication, dual-base
    sel3 = np.zeros((104, 3, 120), f32)
    for base in (0, 64):
        for j in (1, 2):
            sel3[base + ar, j - 1, j * 40 + ar] = 1.0
        for j in range(3):
            sel3[base + ar, 2, j * 40 + ar] = 1.0

    aqkw = np.zeros((128, 4, 4, 208), f32)
    for g in range(4):
        for c in range(4):
            rows = slice(c * 128, (c + 1) * 128)
            aqkw[:, g, c, 0:40] = (MQ[2 * g] / Rq).T[rows]
            aqkw[:, g, c, 64:104] = (MQ[2 * g + 1] / Rq).T[rows]
            aqkw[:, g, c, 104:144] = (MK[2 * g] / Rk).T[rows]
            aqkw[:, g, c, 168:208] = (MK[2 * g + 1] / Rk).T[rows]

    Wv, Wo = np.asarray(inputs["Wv"], f32), np.asarray(inputs["Wo"], f32)
    wvT = np.ascontiguousarray(Wv.T.reshape(4, 128, D).transpose(1, 0, 2))
    woT = np.ascontiguousarray(Wo.T.reshape(4, 128, D).transpose(1, 0, 2))

    h16 = np.float16
    return {
        "aqkw": aqkw.astype(h16),
        "wvT": wvT.astype(h16), "woT": woT.astype(h16),
        "eav": eav.astype(h16), "sel3": sel3.astype(h16),
        "ident": np.eye(128, dtype=h16),
        "woT8": np.ascontiguousarray(
            Wo.T.reshape(H, DH, D).transpose(1, 0, 2)).astype(h16),
    }


def host_prep(inputs, shared, core):
    b, qh = core // 2, core % 2
    qs = qh * NQ
    f32 = np.float32
    h16 = np.float16
    queries = np.asarray(inputs["queries"], f32)
    keys = np.asarray(inputs["keys"], f32)
    values = np.asarray(inputs["values"], f32)
    m = dict(shared)
    xq = queries[b, qs:qs + NQ, :].T.reshape(4, 128, NQ).transpose(1, 0, 2)
    xk = keys[b].T.reshape(4, 128, LK).transpose(1, 0, 2)
    m["xqk"] = np.concatenate([xq, xk], axis=2).astype(h16)
    m["xvT"] = np.ascontiguousarray(
        values[b].T.reshape(4, 128, LK).transpose(1, 0, 2)).astype(h16)
    return m


_NC_CACHE = {}


def _get_nc():
    if "nc" not in _NC_CACHE:
        _NC_CACHE["nc"] = build_program()
    return _NC_CACHE["nc"]


def make_in_maps(**inputs):
    inputs = {k: np.asarray(v) for k, v in inputs.items()}
    shared = _host_shared(inputs)
    return [host_prep(inputs, shared, core) for core in range(8)]


def unshard(results):
    out = np.empty((B, LQ, D), dtype=np.float32)
    for core in range(8):
        b, qh = core // 2, core % 2
        out[b, qh * NQ:(qh + 1) * NQ, :] = results[core]["y"]
    return out


def kernel(**inputs) -> np.ndarray:
    from concourse.bass_utils import run_bass_kernel_spmd

    nc = _get_nc()
    in_maps = make_in_maps(**inputs)
    res = run_bass_kernel_spmd(nc, in_maps, core_ids=list(range(8)))
    return unshard(res.results)


if __name__ == "__main__":
    rng = np.random.default_rng(0)
    demo = {
        "queries": rng.standard_normal((B, LQ, D), dtype=np.float32),
        "keys": rng.standard_normal((B, LQ, D), dtype=np.float32),
        "values": rng.standard_normal((B, LQ, D), dtype=np.float32),
        "Wq": rng.standard_normal((D, D), dtype=np.float32) * 0.05,
        "Wk": rng.standard_normal((D, D), dtype=np.float32) * 0.05,
        "Wv": rng.standard_normal((D, D), dtype=np.float32) * 0.05,
        "Wo": rng.standard_normal((D, D), dtype=np.float32) * 0.05,
        "Aq": rng.standard_normal((AH, DH), dtype=np.float32) * 0.05,
        "Ak": rng.standard_normal((AH, DH), dtype=np.float32) * 0.05,
        "av": rng.standard_normal((AH,), dtype=np.float32) * 0.05,
    }
    out = kernel(**demo)
    print("kernel ran, output:", out.shape, out.dtype)
